# revision 57
# baseline (speedup 1.0000x reference)
"""Local (windowed) attention with shared KV head — TRN2 Bass kernel.

Problem: b=1, L=4096, d_model=1024, n_head=16, d_head=64, w=512.
  qp = (q@Wq)/8; k,v = kv@Wkv; per 512-chunk attention over {prev,self,next}
  chunks with zero-padded edges (softmax includes exp(0)=1 terms for pads);
  out = ctx @ Wo.

Sharding: sequence-parallel over the 8 chunks, one chunk per NeuronCore.
Each core recomputes the K/V projection for its 3-chunk halo (no
collectives). Edge cores receive zero-filled halo slices, which reproduces
the reference's zero-padding exactly (scores 0 -> exp 1 in the softmax).

v2 (this file): all-bf16 datapath (f32 PSUM accumulation), fast softmax
normalization (reciprocal_approx_fast on the PSUM denominator row + DVE
partition-broadcast multiply), PE warm-up matmuls under the initial DMA
wait, q-projection interleaved into the attention loop, v-transposes via
the DMA XBAR instead of the PE.

Per-core dataflow:
  kvp^T = [Wv|Wk]^T @ kv^T            (24 MMs)   -> vT (rows 0:64), kT (64:128)
  k3T2  = kT duplicated to both partition halves (SBUF->SBUF DMA)
  v65   = DMA-transpose(vT) with a ones column appended   ([y,64+1] tiles)
  qp^T  = (Wq/8)^T @ q^T              (64 MMs)   -> 8 tiles [128,512], head pair per tile
  scores: S^T[y,x] per head, row-packed pairs (2 heads share the PE array)
  P^T   = exp(S^T) on ScalarE, PSUM->SBUF bf16, [128,1024] groups
  ctx^T+Z = [v|1]^T @ P^T fused       (M=65: rows 0:64 ctx, row 64 = softmax denom)
  norm  : zinv=recip_approx(Z row); ctxn = ctx * zinv (partition-broadcast)
  out   = ctxn^T-tiles (lhsT) @ Wo    (64 MMs)   -> [512,1024] row-major -> DMA
"""

import numpy as np

B, L, DM, NH, DH, W = 1, 4096, 1024, 16, 64, 512
NCORES = 8
CH = L // NCORES        # 512 tokens per core
YW = 3 * W              # 1536 halo positions
P = 128
NF = DM // P            # 8 feature tiles
NY = YW // P            # 12 y tiles
NPAIR = NH // 2         # 8 head pairs
NGRP = NY // 2          # 6 score groups of 2 y-tiles

_CACHE = {}


def _build():
    import concourse.mybir as mybir
    import concourse.tile as tile
    from concourse import bacc
    from concourse.masks import make_identity
    from contextlib import ExitStack

    F32 = mybir.dt.float32
    F32R = mybir.dt.float32r
    BF16 = mybir.dt.bfloat16
    EXP = mybir.ActivationFunctionType.Exp

    nc = bacc.Bacc("TRN2", target_bir_lowering=False, debug=False)
    QT = nc.dram_tensor("QT", [DM, CH], BF16, kind="ExternalInput")
    KVT = nc.dram_tensor("KVT", [DM, YW], BF16, kind="ExternalInput")
    WQ = nc.dram_tensor("WQ", [DM, DM], BF16, kind="ExternalInput")     # pre-scaled by 1/8
    WVK = nc.dram_tensor("WVK", [DM, P], BF16, kind="ExternalInput")    # [Wv | Wk]
    WO = nc.dram_tensor("WO", [DM, DM], BF16, kind="ExternalInput")
    OUT = nc.dram_tensor("OUT", [CH, DM], F32, kind="ExternalOutput")

    with tile.TileContext(nc) as tc, ExitStack() as ctx:
        perm = ctx.enter_context(tc.tile_pool(name="perm", bufs=1))

        identb = perm.tile([P, P], BF16, tag="identb")
        make_identity(nc, identb[:])
        warmsb = perm.tile([P, W], BF16, tag="warmsb")
        nc.vector.memset(warmsb[:], 1.0)
        zw = perm.tile([1, 16], F32, tag="zw")
        nc.vector.memset(zw[:], 0.0)
        # [1,128] f32r selectors: selA broadcasts zinvA into output partitions
        # 0:64, selB into 64:128 (two accumulating K=1 matmuls build zbc)
        selA = perm.tile([1, P], F32R, tag="selA")
        selB = perm.tile([1, P], F32R, tag="selB")
        nc.vector.memset(selA[:].bitcast(F32), 0.0)
        nc.vector.memset(selA[0:1, 0:64].bitcast(F32), 1.0)
        nc.vector.memset(selB[:].bitcast(F32), 0.0)
        nc.vector.memset(selB[0:1, 64:128].bitcast(F32), 1.0)
        zwo = perm.tile([1, 16], F32, tag="zwo")
        # early exp-table load on ScalarE (runs during the initial DMA wait)
        nc.scalar.activation(zwo[:], zw[:], EXP)

        # --- persistent SBUF tiles
        wvk = [perm.tile([P, P], BF16, tag=f"wvk{f}", name=f"wvk{f}") for f in range(NF)]
        wq = [perm.tile([P, DM], BF16, tag=f"wq{f}", name=f"wq{f}") for f in range(NF)]
        wo = [perm.tile([P, DM], BF16, tag=f"wo{f}", name=f"wo{f}") for f in range(NF)]
        qt = [perm.tile([P, CH], BF16, tag=f"qt{f}", name=f"qt{f}") for f in range(NF)]
        k3T2 = perm.tile([P, YW], BF16, tag="k3T2")
        vTs = perm.tile([64, YW], BF16, tag="vTs")
        v65 = [perm.tile([P, 65], BF16, tag=f"v65_{t}", name=f"v65_{t}") for t in range(NY)]
        qpT = [perm.tile([P, CH], BF16, tag=f"qpT{m}", name=f"qpT{m}") for m in range(NF)]
        ctxn = [perm.tile([P, CH], BF16, tag=f"ctxn{i}", name=f"ctxn{i}") for i in range(NPAIR)]
        zsp = [perm.tile([1, W], F32, tag=f"zsp{h}", name=f"zsp{h}") for h in range(NH)]
        zif = [perm.tile([1, W], F32, tag=f"zif{h}", name=f"zif{h}") for h in range(NH)]
        zir = [perm.tile([1, W], F32R, tag=f"zir{h}", name=f"zir{h}") for h in range(NH)]
        outacc = [perm.tile([P, W], F32, tag=f"oa{j}", name=f"oa{j}")
                  for j in range(8)]

        for f in range(NF):
            nc.sync.dma_start(wvk[f][:], WVK.ap()[P * f:P * (f + 1), :])

        def qproj(m, pool):
            ps = pool.tile([P, CH], F32, tag="mis")
            for f in range(NF):
                nc.tensor.matmul(ps[:], wq[f][:, P * m:P * (m + 1)], qt[f][:],
                                 start=(f == 0), stop=(f == NF - 1))
            with nc.allow_low_precision(reason="bf16 datapath"):
                nc.vector.tensor_copy(qpT[m][:], ps[:])

        with tc.tile_pool(name="kvt", bufs=1) as kvtp, \
             tc.tile_pool(name="warm", bufs=1, space="PSUM") as wmp, \
             tc.tile_pool(name="tpps", bufs=2, space="PSUM") as tpp, \
             tc.tile_pool(name="qpps", bufs=2, space="PSUM") as qpp, \
             tc.tile_pool(name="ph0ps", bufs=2, space="PSUM") as ph0:
            kvt = [kvtp.tile([P, YW], BF16, tag=f"kvt{f}", name=f"kvt{f}") for f in range(NF)]
            # Head loads are DMA-limited: each DGE ring moves ~150GB/s and
            # each dma_start costs ~650ns of its issue queue. Priority order:
            # KVT chunk 0 (sync) + QT (scalar) + first WQ columns (gpsimd)
            # land in parallel; the rest follows; WO is deferred into the
            # attention loop so it doesn't steal head bandwidth.
            for f in range(NF):
                nc.sync.dma_start(kvt[f][:, 0:W], KVT.ap()[P * f:P * (f + 1), 0:W])
            for f in range(NF):
                nc.scalar.dma_start(qt[f][:], QT.ap()[P * f:P * (f + 1), :])
            for f in range(NF):
                nc.gpsimd.dma_start(wq[f][:, 0:2 * P],
                                    WQ.ap()[P * f:P * (f + 1), 0:2 * P])
            for f in range(0, NF, 2):
                nc.sync.dma_start(kvt[f][:, W:], KVT.ap()[P * f:P * (f + 1), W:])
            for f in range(1, NF, 2):
                nc.scalar.dma_start(kvt[f][:, W:], KVT.ap()[P * f:P * (f + 1), W:])
            for f in range(NF):
                nc.gpsimd.dma_start(wq[f][:, 2 * P:],
                                    WQ.ap()[P * f:P * (f + 1), 2 * P:])
            # PE warm-up: dense accumulating matmuls over dummy data keep the
            # HAM activity monitor busy while the KVT DMA lands (K=8/8 sooner)
            wps = wmp.tile([P, W], F32, tag="wps")
            for k in range(6):
                nc.tensor.matmul(wps[:], identb[:], warmsb[:],
                                 start=(k == 0), stop=(k == 5))

            # kv projection: [128,512] psum per n-tile; rows 0:64=vT, 64:128=kT
            def kvchunk(n):
                ps = ph0.tile([P, W], F32, tag="kvp")
                for f in range(NF):
                    nc.tensor.matmul(ps[:], wvk[f][:], kvt[f][:, W * n:W * (n + 1)],
                                     start=(f == 0), stop=(f == NF - 1))
                ns = slice(W * n, W * (n + 1))
                with nc.allow_low_precision(reason="bf16 datapath"):
                    nc.vector.tensor_copy(vTs[:, ns], ps[0:64, :])
                    nc.vector.tensor_copy(k3T2[64:128, ns], ps[64:128, :])
                # duplicate kT into the low partition half per chunk so pair-0
                # scores can start as soon as the first chunks are projected
                nc.gpsimd.dma_start(k3T2[0:64, ns], k3T2[64:128, ns])
                # v65 tiles for this chunk: PE transpose of vT slices
                for t in range(4 * n, 4 * n + 4):
                    tp = tpp.tile([P, 64], BF16, tag="tp")
                    nc.tensor.transpose(tp[:], vTs[:, P * t:P * (t + 1)],
                                        identb[0:64, 0:64])
                    nc.vector.tensor_copy(v65[t][:, 0:64], tp[:])
                    nc.vector.memset(v65[t][:, 64:65], 1.0)

            # chunk 0 first, then the first q projections (pair 0 needs only
            # chunk 0 + qpT[0] to start scoring), then the halo chunks
            kvchunk(0)
            qproj(0, qpp)
            qproj(1, qpp)
            kvchunk(1)
            kvchunk(2)

        # --- attention per head pair; remaining q projections interleaved
        with tc.tile_pool(name="scps", bufs=2, space="PSUM") as scp, \
             tc.tile_pool(name="cxps", bufs=3, space="PSUM") as cxp, \
             tc.tile_pool(name="msps", bufs=1, space="PSUM") as msp, \
             tc.tile_pool(name="pt", bufs=4) as ptp:
            def normalize(i, cxA, cxB):
                # ctxn[i][0:64] = cxA[0:64]/Z_A ; [64:128] = cxB/Z_B
                zbc = msp.tile([P, W], F32, tag="mis")
                for h, cx, sel in ((0, cxA, selA), (1, cxB, selB)):
                    zsh, zih, zrh = zsp[2 * i + h], zif[2 * i + h], zir[2 * i + h]
                    nc.vector.tensor_copy(zsh[:], cx[64:65, :])
                    nc.vector.reciprocal_approx_fast(zih[:], zsh[:])
                    with nc.allow_low_precision(reason="f32r broadcast matmul"):
                        nc.vector.tensor_copy(zrh[:], zih[:])
                    nc.tensor.matmul(zbc[:], sel[:], zrh[:],
                                     start=(h == 0), stop=(h == 1),
                                     tile_position=(0, 0))
                cxs = ptp.tile([P, W], BF16, tag="cbt")
                with nc.allow_low_precision(reason="bf16 datapath"):
                    nc.vector.tensor_copy(cxs[0:64, :], cxA[0:64, :])
                    nc.vector.tensor_copy(cxs[64:128, :], cxB[0:64, :])
                    nc.vector.tensor_mul(ctxn[i][:], cxs[:], zbc[:])

            # PE filler fragments keyed by (pair, group): remaining q
            # projections during pairs 0..3, then the output projection
            # accumulated in three stages (pairs 4/5: heads 0..7, pair 6:
            # 8..11, pair 7: 12,13) so the tail only multiplies pair 7.
            qps = {}

            def qp_frag(m, lo, hi):
                def run():
                    if m not in qps:
                        qps[m] = msp.tile([P, CH], F32, tag="mis",
                                          name=f"qpf{m}")
                    ps = qps[m]
                    for f in range(lo, hi):
                        nc.tensor.matmul(ps[:], wq[f][:, P * m:P * (m + 1)],
                                         qt[f][:], start=(f == 0),
                                         stop=(f == NF - 1))
                    if hi == NF:
                        with nc.allow_low_precision(reason="bf16 datapath"):
                            nc.vector.tensor_copy(qpT[m][:], ps[:])
                        del qps[m]
                return run

            def op_frag(j, lo, hi):
                def run():
                    ps = msp.tile([P, W], F32, tag="mis", name=f"opf{j}_{lo}")
                    x, o = divmod(j, 2)
                    for ii in range(lo, hi):
                        nc.tensor.matmul(ps[:], ctxn[ii][:, P * x:P * (x + 1)],
                                         wo[ii][:, W * o:W * (o + 1)],
                                         start=(ii == lo), stop=(ii == hi - 1))
                    if lo == 0:
                        nc.vector.tensor_copy(outacc[j][:], ps[:])
                    else:
                        nc.vector.tensor_add(outacc[j][:], ps[:], outacc[j][:])
                return run

            sched = {}
            for i in range(6):              # q projections m=2..7, pairs 0..5
                sched.setdefault((i, 2), []).append(qp_frag(i + 2, 0, 4))
                sched.setdefault((i, 4), []).append(qp_frag(i + 2, 4, NF))
            for j in range(8):              # out-proj stage 1: heads 0..7
                sched.setdefault((4 + j // 4, 1 + j % 4), []).append(
                    op_frag(j, 0, 4))
            g67 = [0, 1, 1, 2, 3, 3, 4, 5]
            for j in range(8):              # out-proj stage 2: heads 8..11
                sched.setdefault((6, g67[j]), []).append(op_frag(j, 4, 6))
            for j in range(8):              # out-proj stage 3: heads 12,13
                sched.setdefault((7, g67[j]), []).append(op_frag(j, 6, 7))

            pend = None
            for i in range(NPAIR):
                cxA = cxp.tile([P, W], F32, tag="cx")
                cxB = cxp.tile([P, W], F32, tag="cx")
                for g in range(NGRP):
                    scA = scp.tile([P, 2 * W], F32, tag="sc")
                    scB = scp.tile([P, 2 * W], F32, tag="sc")
                    for t in range(2):
                        y = 2 * g + t
                        ys = slice(P * y, P * (y + 1))
                        ts_ = slice(W * t, W * (t + 1))
                        nc.tensor.matmul(scA[:, ts_], k3T2[0:64, ys],
                                         qpT[i][0:64, :], start=True, stop=True,
                                         tile_position=(0, 0))
                        nc.tensor.matmul(scB[:, ts_], k3T2[64:128, ys],
                                         qpT[i][64:128, :], start=True, stop=True,
                                         tile_position=(64, 0))
                    if g == 0 and pend is not None:
                        # deferred normalize: issued after the next pair's
                        # first score MMs so ACT never waits at pair boundary
                        normalize(*pend)
                        pend = None
                    pA = ptp.tile([P, 2 * W], BF16, tag="pt")
                    pB = ptp.tile([P, 2 * W], BF16, tag="pt")
                    nc.scalar.activation(pA[:], scA[:], EXP)
                    nc.scalar.activation(pB[:], scB[:], EXP)
                    for t in range(2):
                        y = 2 * g + t
                        ts_ = slice(W * t, W * (t + 1))
                        st = (g == 0 and t == 0)
                        sp = (g == NGRP - 1 and t == 1)
                        nc.tensor.matmul(cxA[0:65, :], v65[y][:], pA[:, ts_],
                                         start=st, stop=sp)
                        nc.tensor.matmul(cxB[0:65, :], v65[y][:], pB[:, ts_],
                                         start=st, stop=sp)
                    for fr in sched.get((i, g), ()):
                        fr()
                    if i == 1 and g == 0:
                        # deferred WO load: after the head's critical DMAs
                        for f in range(NF):
                            nc.sync.dma_start(wo[f][:],
                                              WO.ap()[P * f:P * (f + 1), :])
                pend = (i, cxA, cxB)
            normalize(*pend)

        # --- output projection tail: only pair 7 + the accumulated partials
        with tc.tile_pool(name="opps", bufs=4, space="PSUM") as opp, \
             tc.tile_pool(name="osb", bufs=4) as osb:
            for x in range(4):
                xs = slice(P * x, P * (x + 1))
                for o in range(2):
                    os_ = slice(W * o, W * (o + 1))
                    ps = opp.tile([P, W], F32, tag="op")
                    nc.tensor.matmul(ps[:], ctxn[7][:, xs], wo[7][:, os_],
                                     start=True, stop=True)
                    ot = osb.tile([P, W], F32, tag="os")
                    nc.vector.tensor_add(ot[:], ps[:], outacc[2 * x + o][:])
                    eng = nc.sync if (2 * x + o) % 2 == 0 else nc.gpsimd
                    eng.dma_start(OUT.ap()[xs, os_], ot[:])

    nc.compile()
    return nc


def _get_nc():
    if "nc" not in _CACHE:
        _CACHE["nc"] = _build()
    return _CACHE["nc"]


def kernel(q, kv, Wq, Wkv, Wo, w=None, _trace=False):
    import ml_dtypes
    from concourse import bass_utils

    BF = ml_dtypes.bfloat16

    q = np.asarray(q, np.float32).reshape(L, DM)
    kv = np.asarray(kv, np.float32).reshape(L, DM)
    Wq = np.asarray(Wq, np.float32)
    Wkv = np.asarray(Wkv, np.float32)
    Wo = np.asarray(Wo, np.float32)

    qT = np.ascontiguousarray(q.T).astype(BF)            # [DM, L]
    kvT = np.ascontiguousarray(kv.T).astype(BF)          # [DM, L]
    WQs = np.ascontiguousarray(Wq / np.sqrt(DH)).astype(BF)  # fold 1/sqrt(d_head)
    WVK = np.ascontiguousarray(
        np.concatenate([Wkv[:, DH:], Wkv[:, :DH]], axis=1)).astype(BF)  # [Wv | Wk]
    WOc = np.ascontiguousarray(Wo).astype(BF)

    in_maps = []
    for c in range(NCORES):
        kvt_c = np.zeros((DM, YW), BF)
        lo = (c - 1) * CH
        hi = (c + 2) * CH
        src_lo, src_hi = max(lo, 0), min(hi, L)
        dst_lo = src_lo - lo
        kvt_c[:, dst_lo:dst_lo + (src_hi - src_lo)] = kvT[:, src_lo:src_hi]
        in_maps.append({
            "QT": np.ascontiguousarray(qT[:, c * CH:(c + 1) * CH]),
            "KVT": kvt_c,
            "WQ": WQs,
            "WVK": WVK,
            "WO": WOc,
        })

    nc = _get_nc()
    res = bass_utils.run_bass_kernel_spmd(
        nc, in_maps, core_ids=list(range(NCORES)), trace=_trace)
    if _trace:
        _CACHE["last_result"] = res

    out = np.concatenate([r["OUT"] for r in res.results], axis=0)
    return out.reshape(B, L, DM).astype(np.float32)


# revision 60
# speedup vs baseline: 1.2006x; 1.2006x over previous
"""Local (windowed) attention with shared KV head — TRN2 Bass kernel.

Problem: b=1, L=4096, d_model=1024, n_head=16, d_head=64, w=512.
  qp = (q@Wq)/8; k,v = kv@Wkv; per 512-chunk attention over {prev,self,next}
  chunks with zero-padded edges (softmax includes exp(0)=1 terms for pads);
  out = ctx @ Wo.

Sharding: sequence-parallel over the 8 chunks, one chunk per NeuronCore.
Each core recomputes the K/V projection for its 3-chunk halo (no
collectives). Edge cores receive zero-filled halo slices, which reproduces
the reference's zero-padding exactly (scores 0 -> exp 1 in the softmax).

v2 (this file): all-bf16 datapath (f32 PSUM accumulation), fast softmax
normalization (reciprocal_approx_fast on the PSUM denominator row + DVE
partition-broadcast multiply), PE warm-up matmuls under the initial DMA
wait, q-projection interleaved into the attention loop, v-transposes via
the DMA XBAR instead of the PE.

Per-core dataflow:
  kvp^T = [Wv|Wk]^T @ kv^T            (24 MMs)   -> vT (rows 0:64), kT (64:128)
  k3T2  = kT duplicated to both partition halves (SBUF->SBUF DMA)
  v65   = DMA-transpose(vT) with a ones column appended   ([y,64+1] tiles)
  qp^T  = (Wq/8)^T @ q^T              (64 MMs)   -> 8 tiles [128,512], head pair per tile
  scores: S^T[y,x] per head, row-packed pairs (2 heads share the PE array)
  P^T   = exp(S^T) on ScalarE, PSUM->SBUF bf16, [128,1024] groups
  ctx^T+Z = [v|1]^T @ P^T fused       (M=65: rows 0:64 ctx, row 64 = softmax denom)
  norm  : zinv=recip_approx(Z row); ctxn = ctx * zinv (partition-broadcast)
  out   = ctxn^T-tiles (lhsT) @ Wo    (64 MMs)   -> [512,1024] row-major -> DMA
"""

import numpy as np

B, L, DM, NH, DH, W = 1, 4096, 1024, 16, 64, 512
NCORES = 8
CH = L // NCORES        # 512 tokens per core
YW = 3 * W              # 1536 halo positions
P = 128
NF = DM // P            # 8 feature tiles
NY = YW // P            # 12 y tiles
NPAIR = NH // 2         # 8 head pairs
NGRP = NY // 2          # 6 score groups of 2 y-tiles

_CACHE = {}


def _build():
    import concourse.mybir as mybir
    import concourse.tile as tile
    from concourse import bacc
    from concourse.masks import make_identity
    from contextlib import ExitStack

    F32 = mybir.dt.float32
    F32R = mybir.dt.float32r
    BF16 = mybir.dt.bfloat16
    EXP = mybir.ActivationFunctionType.Exp

    nc = bacc.Bacc("TRN2", target_bir_lowering=False, debug=False)
    QT = nc.dram_tensor("QT", [DM, CH], BF16, kind="ExternalInput")
    KVT = nc.dram_tensor("KVT", [DM, YW], BF16, kind="ExternalInput")
    WQ = nc.dram_tensor("WQ", [DM, DM], BF16, kind="ExternalInput")     # pre-scaled by 1/8
    WVK = nc.dram_tensor("WVK", [DM, P], BF16, kind="ExternalInput")    # [Wv | Wk]
    WO = nc.dram_tensor("WO", [DM, DM], BF16, kind="ExternalInput")
    OUT = nc.dram_tensor("OUT", [CH, DM], F32, kind="ExternalOutput")

    with tile.TileContext(nc) as tc, ExitStack() as ctx:
        perm = ctx.enter_context(tc.tile_pool(name="perm", bufs=1))

        identb = perm.tile([P, P], BF16, tag="identb")
        make_identity(nc, identb[:])
        warmsb = perm.tile([P, W], BF16, tag="warmsb")
        nc.vector.memset(warmsb[:], 1.0)
        zw = perm.tile([1, 16], F32, tag="zw")
        nc.vector.memset(zw[:], 0.0)
        # [1,128] f32r selectors: selA broadcasts zinvA into output partitions
        # 0:64, selB into 64:128 (two accumulating K=1 matmuls build zbc)
        selA = perm.tile([1, P], F32R, tag="selA")
        selB = perm.tile([1, P], F32R, tag="selB")
        nc.vector.memset(selA[:].bitcast(F32), 0.0)
        nc.vector.memset(selA[0:1, 0:64].bitcast(F32), 1.0)
        nc.vector.memset(selB[:].bitcast(F32), 0.0)
        nc.vector.memset(selB[0:1, 64:128].bitcast(F32), 1.0)
        zwo = perm.tile([1, 16], F32, tag="zwo")
        # early exp-table load on ScalarE (runs during the initial DMA wait)
        nc.scalar.activation(zwo[:], zw[:], EXP)

        # --- persistent SBUF tiles
        wvk = [perm.tile([P, P], BF16, tag=f"wvk{f}", name=f"wvk{f}") for f in range(NF)]
        wq = [perm.tile([P, DM], BF16, tag=f"wq{f}", name=f"wq{f}") for f in range(NF)]
        wo = [perm.tile([P, DM], BF16, tag=f"wo{f}", name=f"wo{f}") for f in range(NF)]
        qt = [perm.tile([P, CH], BF16, tag=f"qt{f}", name=f"qt{f}") for f in range(NF)]
        k3T2 = perm.tile([P, YW], BF16, tag="k3T2")
        vTs = perm.tile([64, YW], BF16, tag="vTs")
        v65 = [perm.tile([P, 65], BF16, tag=f"v65_{t}", name=f"v65_{t}") for t in range(NY)]
        qpT = [perm.tile([P, CH], BF16, tag=f"qpT{m}", name=f"qpT{m}") for m in range(NF)]
        ctxn = [perm.tile([P, CH], BF16, tag=f"ctxn{i}", name=f"ctxn{i}") for i in range(NPAIR)]
        zsp = [perm.tile([1, W], F32, tag=f"zsp{h}", name=f"zsp{h}") for h in range(NH)]
        zif = [perm.tile([1, W], F32, tag=f"zif{h}", name=f"zif{h}") for h in range(NH)]
        zir = [perm.tile([1, W], F32R, tag=f"zir{h}", name=f"zir{h}") for h in range(NH)]
        outacc = [perm.tile([P, W], F32, tag=f"oa{j}", name=f"oa{j}")
                  for j in range(8)]

        for f in range(NF):
            nc.sync.dma_start(wvk[f][:], WVK.ap()[P * f:P * (f + 1), :])

        def qproj(m, pool):
            ps = pool.tile([P, CH], F32, tag="mis")
            for f in range(NF):
                nc.tensor.matmul(ps[:], wq[f][:, P * m:P * (m + 1)], qt[f][:],
                                 start=(f == 0), stop=(f == NF - 1))
            with nc.allow_low_precision(reason="bf16 datapath"):
                nc.vector.tensor_copy(qpT[m][:], ps[:])

        with tc.tile_pool(name="kvt", bufs=1) as kvtp, \
             tc.tile_pool(name="warm", bufs=1, space="PSUM") as wmp, \
             tc.tile_pool(name="tpps", bufs=2, space="PSUM") as tpp, \
             tc.tile_pool(name="qpps", bufs=2, space="PSUM") as qpp, \
             tc.tile_pool(name="ph0ps", bufs=2, space="PSUM") as ph0:
            kvt = [kvtp.tile([P, YW], BF16, tag=f"kvt{f}", name=f"kvt{f}") for f in range(NF)]
            # Head loads are DMA-limited: each DGE ring moves ~150GB/s and
            # each dma_start costs ~650ns of its issue queue. Priority order:
            # KVT chunk 0 (sync) + QT (scalar) + first WQ columns (gpsimd)
            # land in parallel; the rest follows; WO is deferred into the
            # attention loop so it doesn't steal head bandwidth.
            for f in range(NF):
                nc.sync.dma_start(kvt[f][:, 0:W], KVT.ap()[P * f:P * (f + 1), 0:W])
            for f in range(NF):
                nc.scalar.dma_start(qt[f][:], QT.ap()[P * f:P * (f + 1), :])
            for f in range(NF):
                nc.gpsimd.dma_start(wq[f][:, 0:2 * P],
                                    WQ.ap()[P * f:P * (f + 1), 0:2 * P])
            for f in range(0, NF, 2):
                nc.sync.dma_start(kvt[f][:, W:], KVT.ap()[P * f:P * (f + 1), W:])
            for f in range(1, NF, 2):
                nc.scalar.dma_start(kvt[f][:, W:], KVT.ap()[P * f:P * (f + 1), W:])
            for f in range(NF):
                nc.gpsimd.dma_start(wq[f][:, 2 * P:],
                                    WQ.ap()[P * f:P * (f + 1), 2 * P:])
            # PE warm-up: dense accumulating matmuls over dummy data keep the
            # HAM activity monitor busy while the KVT DMA lands (K=8/8 sooner)
            wps = wmp.tile([P, W], F32, tag="wps")
            for k in range(6):
                nc.tensor.matmul(wps[:], identb[:], warmsb[:],
                                 start=(k == 0), stop=(k == 5))

            # kv projection: [128,512] psum per n-tile; rows 0:64=vT, 64:128=kT
            def kvchunk(n):
                ps = ph0.tile([P, W], F32, tag="kvp")
                for f in range(NF):
                    nc.tensor.matmul(ps[:], wvk[f][:], kvt[f][:, W * n:W * (n + 1)],
                                     start=(f == 0), stop=(f == NF - 1))
                ns = slice(W * n, W * (n + 1))
                with nc.allow_low_precision(reason="bf16 datapath"):
                    nc.vector.tensor_copy(vTs[:, ns], ps[0:64, :])
                    nc.vector.tensor_copy(k3T2[64:128, ns], ps[64:128, :])
                # duplicate kT into the low partition half per chunk so pair-0
                # scores can start as soon as the first chunks are projected
                nc.gpsimd.dma_start(k3T2[0:64, ns], k3T2[64:128, ns])
                # v65 tiles for this chunk: PE transpose of vT slices
                for t in range(4 * n, 4 * n + 4):
                    tp = tpp.tile([P, 64], BF16, tag="tp")
                    nc.tensor.transpose(tp[:], vTs[:, P * t:P * (t + 1)],
                                        identb[0:64, 0:64])
                    nc.vector.tensor_copy(v65[t][:, 0:64], tp[:])
                    nc.vector.memset(v65[t][:, 64:65], 1.0)

            # chunk 0 first, then the first q projections (pair 0 needs only
            # chunk 0 + qpT[0] to start scoring), then the halo chunks
            kvchunk(0)
            qproj(0, qpp)
            qproj(1, qpp)
            kvchunk(1)
            kvchunk(2)

        # --- attention per head pair; remaining q projections interleaved
        with tc.tile_pool(name="scps", bufs=2, space="PSUM") as scp, \
             tc.tile_pool(name="cxps", bufs=3, space="PSUM") as cxp, \
             tc.tile_pool(name="msps", bufs=1, space="PSUM") as msp, \
             tc.tile_pool(name="pt", bufs=4) as ptp:
            def normalize(i, cxA, cxB):
                # ctxn[i][0:64] = cxA[0:64]/Z_A ; [64:128] = cxB/Z_B
                zbc = msp.tile([P, W], F32, tag="mis")
                for h, cx, sel in ((0, cxA, selA), (1, cxB, selB)):
                    zsh, zih, zrh = zsp[2 * i + h], zif[2 * i + h], zir[2 * i + h]
                    nc.vector.tensor_copy(zsh[:], cx[64:65, :])
                    nc.vector.reciprocal_approx_fast(zih[:], zsh[:])
                    with nc.allow_low_precision(reason="f32r broadcast matmul"):
                        nc.vector.tensor_copy(zrh[:], zih[:])
                    nc.tensor.matmul(zbc[:], sel[:], zrh[:],
                                     start=(h == 0), stop=(h == 1),
                                     tile_position=(0, 0))
                cxs = ptp.tile([P, W], BF16, tag="cbt")
                with nc.allow_low_precision(reason="bf16 datapath"):
                    nc.vector.tensor_copy(cxs[0:64, :], cxA[0:64, :])
                    nc.vector.tensor_copy(cxs[64:128, :], cxB[0:64, :])
                    nc.vector.tensor_mul(ctxn[i][:], cxs[:], zbc[:])

            # PE filler fragments keyed by (pair, group): remaining q
            # projections during pairs 0..3, then the output projection
            # accumulated in three stages (pairs 4/5: heads 0..7, pair 6:
            # 8..11, pair 7: 12,13) so the tail only multiplies pair 7.
            qps = {}

            def qp_frag(m, lo, hi):
                def run():
                    if m not in qps:
                        qps[m] = msp.tile([P, CH], F32, tag="mis",
                                          name=f"qpf{m}")
                    ps = qps[m]
                    for f in range(lo, hi):
                        nc.tensor.matmul(ps[:], wq[f][:, P * m:P * (m + 1)],
                                         qt[f][:], start=(f == 0),
                                         stop=(f == NF - 1))
                    if hi == NF:
                        with nc.allow_low_precision(reason="bf16 datapath"):
                            nc.vector.tensor_copy(qpT[m][:], ps[:])
                        del qps[m]
                return run

            def op_frag(j, lo, hi):
                def run():
                    if ('o', j) not in qps:
                        qps[('o', j)] = msp.tile([P, W], F32, tag="mis",
                                                 name=f"opf{j}")
                    ps = qps[('o', j)]
                    x, o = divmod(j, 2)
                    for ii in range(lo, hi):
                        nc.tensor.matmul(ps[:], ctxn[ii][:, P * x:P * (x + 1)],
                                         wo[ii][:, W * o:W * (o + 1)],
                                         start=(ii == 0), stop=(ii == 5))
                    if hi == 6:
                        nc.vector.tensor_copy(outacc[j][:], ps[:])
                        del qps[('o', j)]
                return run

            sched = {}
            for i in range(6):              # q projections m=2..7, pairs 0..5
                sched.setdefault((i, 2), []).append(qp_frag(i + 2, 0, 4))
                sched.setdefault((i, 4), []).append(qp_frag(i + 2, 4, NF))
            # partial out-proj (heads 0..11) over pairs 6,7: one 3-MM frag
            # per group; j covers two adjacent groups, closed within a pair
            opf = [op_frag(j, lo, lo + 3) for j in range(8) for lo in (0, 3)]
            for k in range(12):
                sched.setdefault((6 + k // 6, k % 6), []).append(opf[k])
            optail = opf[12:]

            pend = None
            for i in range(NPAIR):
                cxA = cxp.tile([P, W], F32, tag="cx")
                cxB = cxp.tile([P, W], F32, tag="cx")
                for g in range(NGRP):
                    scA = scp.tile([P, 2 * W], F32, tag="sc")
                    scB = scp.tile([P, 2 * W], F32, tag="sc")
                    for t in range(2):
                        y = 2 * g + t
                        ys = slice(P * y, P * (y + 1))
                        ts_ = slice(W * t, W * (t + 1))
                        nc.tensor.matmul(scA[:, ts_], k3T2[0:64, ys],
                                         qpT[i][0:64, :], start=True, stop=True,
                                         tile_position=(0, 0))
                        nc.tensor.matmul(scB[:, ts_], k3T2[64:128, ys],
                                         qpT[i][64:128, :], start=True, stop=True,
                                         tile_position=(64, 0))
                    if g == 0 and pend is not None:
                        # deferred normalize: issued after the next pair's
                        # first score MMs so ACT never waits at pair boundary
                        normalize(*pend)
                        pend = None
                    pA = ptp.tile([P, 2 * W], BF16, tag="pt")
                    pB = ptp.tile([P, 2 * W], BF16, tag="pt")
                    nc.scalar.activation(pA[:], scA[:], EXP)
                    nc.scalar.activation(pB[:], scB[:], EXP)
                    for t in range(2):
                        y = 2 * g + t
                        ts_ = slice(W * t, W * (t + 1))
                        st = (g == 0 and t == 0)
                        sp = (g == NGRP - 1 and t == 1)
                        nc.tensor.matmul(cxA[0:65, :], v65[y][:], pA[:, ts_],
                                         start=st, stop=sp)
                        nc.tensor.matmul(cxB[0:65, :], v65[y][:], pB[:, ts_],
                                         start=st, stop=sp)
                    for fr in sched.get((i, g), ()):
                        fr()
                    if i == 1 and g == 0:
                        # deferred WO load: after the head's critical DMAs
                        for f in range(NF):
                            nc.sync.dma_start(wo[f][:],
                                              WO.ap()[P * f:P * (f + 1), :])
                pend = (i, cxA, cxB)
            normalize(*pend)
            for fr in optail:
                fr()

        # --- output projection tail: only pair 7 + the accumulated partials
        with tc.tile_pool(name="opps", bufs=4, space="PSUM") as opp, \
             tc.tile_pool(name="osb", bufs=4) as osb:
            for x in range(4):
                xs = slice(P * x, P * (x + 1))
                for o in range(2):
                    os_ = slice(W * o, W * (o + 1))
                    ps = opp.tile([P, W], F32, tag="op")
                    for i in (6, 7):
                        nc.tensor.matmul(ps[:], ctxn[i][:, xs], wo[i][:, os_],
                                         start=(i == 6), stop=(i == 7))
                    ot = osb.tile([P, W], F32, tag="os")
                    nc.vector.tensor_add(ot[:], ps[:], outacc[2 * x + o][:])
                    eng = nc.sync if (2 * x + o) % 2 == 0 else nc.gpsimd
                    eng.dma_start(OUT.ap()[xs, os_], ot[:])

    nc.compile()
    return nc


def _get_nc():
    if "nc" not in _CACHE:
        _CACHE["nc"] = _build()
    return _CACHE["nc"]


def kernel(q, kv, Wq, Wkv, Wo, w=None, _trace=False):
    import ml_dtypes
    from concourse import bass_utils

    BF = ml_dtypes.bfloat16

    q = np.asarray(q, np.float32).reshape(L, DM)
    kv = np.asarray(kv, np.float32).reshape(L, DM)
    Wq = np.asarray(Wq, np.float32)
    Wkv = np.asarray(Wkv, np.float32)
    Wo = np.asarray(Wo, np.float32)

    qT = np.ascontiguousarray(q.T).astype(BF)            # [DM, L]
    kvT = np.ascontiguousarray(kv.T).astype(BF)          # [DM, L]
    WQs = np.ascontiguousarray(Wq / np.sqrt(DH)).astype(BF)  # fold 1/sqrt(d_head)
    WVK = np.ascontiguousarray(
        np.concatenate([Wkv[:, DH:], Wkv[:, :DH]], axis=1)).astype(BF)  # [Wv | Wk]
    WOc = np.ascontiguousarray(Wo).astype(BF)

    in_maps = []
    for c in range(NCORES):
        kvt_c = np.zeros((DM, YW), BF)
        lo = (c - 1) * CH
        hi = (c + 2) * CH
        src_lo, src_hi = max(lo, 0), min(hi, L)
        dst_lo = src_lo - lo
        kvt_c[:, dst_lo:dst_lo + (src_hi - src_lo)] = kvT[:, src_lo:src_hi]
        in_maps.append({
            "QT": np.ascontiguousarray(qT[:, c * CH:(c + 1) * CH]),
            "KVT": kvt_c,
            "WQ": WQs,
            "WVK": WVK,
            "WO": WOc,
        })

    nc = _get_nc()
    res = bass_utils.run_bass_kernel_spmd(
        nc, in_maps, core_ids=list(range(NCORES)), trace=_trace)
    if _trace:
        _CACHE["last_result"] = res

    out = np.concatenate([r["OUT"] for r in res.results], axis=0)
    return out.reshape(B, L, DM).astype(np.float32)
